# revision 13
# baseline (speedup 1.0000x reference)
"""IPLS.partial_fit Trainium2 kernel (8 NeuronCores).

Strategy: the 64-step latent recurrence only consumes a handful of scalars per
step (dots/norms of the evolving xc/yc against fixed rows of Wz/Cz/P).  Since
xc_i / yc_i stay inside span{xc0, P rows} / span{yc0, Cz rows}, every scalar is
a quadratic form of small Gram matrices:

  phase 1 (device, n_feat sharded 8-way): G=[xc0;P][xc0;P]^T, Wx=[xc0;P]Wz^T,
          sw=rownorm2(Wz) as per-core partials; Gy=[yc0;Cz][yc0;Cz]^T (full).
  phase 2 (host, microseconds): run the sequential recurrence on 65-vectors in
          fp64, producing coefficient matrices Cx, Cy and per-step scalars.
  phase 3 (device, sharded): rank-65 updates as matmuls:
          Wz_new = Wz + (diag(u1) Cx) V,  P_new = P + (diag(t2) Cx) V,
          Cz_new = Cz + (diag(t2) Cy) Vy.

No per-step collectives; both device kernels are memory-bound streams.
"""
import os
import sys

import numpy as np

for _p in ("/opt/trn_rl_repo", os.path.expanduser("~/.axon_site/_ro/trn_rl_repo")):
    if os.path.isdir(_p) and _p not in sys.path:
        sys.path.insert(0, _p)

import concourse.bass as bass  # noqa: E402
import concourse.bacc as bacc  # noqa: E402
import concourse.tile as tile  # noqa: E402
from concourse import mybir  # noqa: E402
from concourse.bass_utils import run_bass_kernel_spmd  # noqa: E402

NF = 131072
NT = 2048
NL = 64
NV = NL + 1
NCORES = 8
SH = NF // NCORES        # 16384 features per core
SHT = NT // NCORES       # 256 targets per core
NCHUNK = SH // 128       # 128 transpose chunks (phase 1)
NYCHUNK = NT // 128      # 16 y-side chunks
LOADCH = 16              # DMA load chunks for the big matrices
LCW = SH // LOADCH       # 1024 columns per load chunk
EPS = 1e-7
F32 = mybir.dt.float32

_K1 = None
_K2 = None
_PROFILE = {"k1_ns": None, "k2_ns": None}


def _new_bass():
    return bacc.Bacc("TRN2", target_bir_lowering=False, debug=False,
                     num_devices=NCORES)


def _build_k1():
    """Per core: V=[xc0_sh;P_sh] [65,SH], Wz_sh [64,SH], Vy=[yc0;Cz] [65,NT].
    Outputs: gwx [65,129] = [G | Wx] partials, sw [64,1] partial, gy [65,65]."""
    nc = _new_bass()
    v_d = nc.dram_tensor("v", [NV, SH], F32, kind="ExternalInput")
    wz_d = nc.dram_tensor("wz", [NL, SH], F32, kind="ExternalInput")
    vy_d = nc.dram_tensor("vy", [NV, NT], F32, kind="ExternalInput")
    gwx_d = nc.dram_tensor("gwx", [NV, NV + NL], F32, kind="ExternalOutput")
    gy_d = nc.dram_tensor("gy", [NV, NV], F32, kind="ExternalOutput")
    sw_d = nc.dram_tensor("sw", [NL, 1], F32, kind="ExternalOutput")
    ident_d = nc.inline_tensor(np.eye(128, dtype=np.float32), "ident")

    with tile.TileContext(nc) as tc:
        with tc.tile_pool(name="big", bufs=1) as big, \
             tc.tile_pool(name="work", bufs=4) as work, \
             tc.tile_pool(name="outp", bufs=1) as outp, \
             tc.tile_pool(name="psum", bufs=3, space="PSUM") as psum, \
             tc.tile_pool(name="acc", bufs=1, space="PSUM") as acc:

            id_sb = big.tile([128, 128], F32)
            nc.sync.dma_start(id_sb[:], ident_d[:])

            v_tiles = []
            wz_tiles = []
            for i in range(LOADCH):
                vt = big.tile([NV, LCW], F32, tag=f"v{i}")
                nc.sync.dma_start(vt[:], v_d[:, i * LCW:(i + 1) * LCW])
                v_tiles.append(vt)
                wt = big.tile([NL, LCW], F32, tag=f"w{i}")
                nc.sync.dma_start(wt[:], wz_d[:, i * LCW:(i + 1) * LCW])
                wz_tiles.append(wt)
            vy_sb = big.tile([NV, NT], F32)
            nc.sync.dma_start(vy_sb[:], vy_d[:])

            gwx_ps = acc.tile([NV, NV + NL], F32)
            gy_ps = acc.tile([NV, NV], F32)

            per_tile = LCW // 128  # 8 transpose chunks per load chunk
            for c in range(NCHUNK):
                vt_src = v_tiles[c // per_tile]
                wz_src = wz_tiles[c // per_tile]
                off = (c % per_tile) * 128
                sl = slice(off, off + 128)
                vt_ps = psum.tile([128, NV], F32, tag="vt")
                nc.tensor.transpose(vt_ps[:], vt_src[:, sl], id_sb[0:NV, 0:NV])
                wzt_ps = psum.tile([128, NL], F32, tag="wzt")
                nc.tensor.transpose(wzt_ps[:], wz_src[:, sl], id_sb[0:NL, 0:NL])
                vwt = work.tile([128, NV + NL], F32, tag="vwt")
                nc.vector.tensor_copy(vwt[:, 0:NV], vt_ps[:])
                nc.vector.tensor_copy(vwt[:, NV:NV + NL], wzt_ps[:])
                nc.tensor.matmul(gwx_ps[:], vwt[:, 0:NV], vwt[:],
                                 start=(c == 0), stop=(c == NCHUNK - 1))

            for c in range(NYCHUNK):
                sl = slice(c * 128, (c + 1) * 128)
                vyt_ps = psum.tile([128, NV], F32, tag="vt")
                nc.tensor.transpose(vyt_ps[:], vy_sb[:, sl], id_sb[0:NV, 0:NV])
                vyt = work.tile([128, NV], F32, tag="vyt")
                nc.vector.tensor_copy(vyt[:], vyt_ps[:])
                nc.tensor.matmul(gy_ps[:], vyt[:], vyt[:],
                                 start=(c == 0), stop=(c == NYCHUNK - 1))

            # sw = rowwise sum of Wz^2 (ACT engine, otherwise idle)
            sw_parts = outp.tile([NL, LOADCH], F32)
            for i in range(LOADCH):
                sq = work.tile([NL, LCW], F32, tag="sq")
                nc.scalar.activation(sq[:], wz_tiles[i][:],
                                     mybir.ActivationFunctionType.Square,
                                     accum_out=sw_parts[:, i:i + 1])
            sw_sb = outp.tile([NL, 1], F32)
            nc.vector.tensor_reduce(sw_sb[:], sw_parts[:],
                                    mybir.AxisListType.X, mybir.AluOpType.add)

            gwx_sb = outp.tile([NV, NV + NL], F32)
            nc.vector.tensor_copy(gwx_sb[:], gwx_ps[:])
            gy_sb = outp.tile([NV, NV], F32)
            nc.vector.tensor_copy(gy_sb[:], gy_ps[:])
            nc.sync.dma_start(gwx_d[:], gwx_sb[:])
            nc.sync.dma_start(gy_d[:], gy_sb[:])
            nc.sync.dma_start(sw_d[:], sw_sb[:])

    nc.compile()
    if not nc.is_finalized():
        nc.finalize()
    return nc


def _build_k2():
    """Per core: apply the rank-65 updates.
    out[0:64]  = Awz @ V -> Wz_new = Wz + .
    out[64:128]= Ap  @ V -> P_new  = P + .
    y-side: Cz_new = Cz + Acz @ Vy (target-sharded)."""
    nc = _new_bass()
    v_d = nc.dram_tensor("v", [NV, SH], F32, kind="ExternalInput")
    wz_d = nc.dram_tensor("wz", [NL, SH], F32, kind="ExternalInput")
    vy_d = nc.dram_tensor("vy", [NV, SHT], F32, kind="ExternalInput")
    at_d = nc.dram_tensor("at", [NV, 128], F32, kind="ExternalInput")
    ayt_d = nc.dram_tensor("ayt", [NV, NL], F32, kind="ExternalInput")
    wzn_d = nc.dram_tensor("wzn", [NL, SH], F32, kind="ExternalOutput")
    pn_d = nc.dram_tensor("pn", [NL, SH], F32, kind="ExternalOutput")
    czn_d = nc.dram_tensor("czn", [NL, SHT], F32, kind="ExternalOutput")

    with tile.TileContext(nc) as tc:
        with tc.tile_pool(name="big", bufs=1) as big, \
             tc.tile_pool(name="outs", bufs=3) as outs, \
             tc.tile_pool(name="psum", bufs=3, space="PSUM") as psum:

            at_sb = big.tile([NV, 128], F32)
            nc.sync.dma_start(at_sb[:], at_d[:])
            ayt_sb = big.tile([NV, NL], F32)
            nc.sync.dma_start(ayt_sb[:], ayt_d[:])
            vy_sb = big.tile([NV, SHT], F32)
            nc.sync.dma_start(vy_sb[:], vy_d[:])

            v_tiles = []
            wz_tiles = []
            for i in range(LOADCH):
                vt = big.tile([NV, LCW], F32, tag=f"v{i}")
                nc.sync.dma_start(vt[:], v_d[:, i * LCW:(i + 1) * LCW])
                v_tiles.append(vt)
                wt = big.tile([NL, LCW], F32, tag=f"w{i}")
                nc.sync.dma_start(wt[:], wz_d[:, i * LCW:(i + 1) * LCW])
                wz_tiles.append(wt)

            # y-side first (tiny); Vy rows are [Cz; yc0] so the Cz operand
            # of the add starts at partition 0
            yt_ps = psum.tile([NL, SHT], F32, tag="ypsum")
            nc.tensor.matmul(yt_ps[:], ayt_sb[:], vy_sb[:], start=True, stop=True)
            cz_o = outs.tile([NL, SHT], F32, tag="czo")
            nc.vector.tensor_add(cz_o[:], yt_ps[:], vy_sb[0:NL, :])
            nc.sync.dma_start(czn_d[:], cz_o[:])

            per_tile = LCW // 512  # 2 matmul slices per load chunk
            for i in range(LOADCH):
                wz_o = outs.tile([NL, LCW], F32, tag="wzo")
                p_o = outs.tile([NL, LCW], F32, tag="po")
                for j in range(per_tile):
                    sl = slice(j * 512, (j + 1) * 512)
                    xt_ps = psum.tile([128, 512], F32, tag="xpsum")
                    nc.tensor.matmul(xt_ps[:], at_sb[:], v_tiles[i][:, sl],
                                     start=True, stop=True)
                    nc.vector.tensor_add(wz_o[:, sl], xt_ps[0:NL, :],
                                         wz_tiles[i][:, sl])
                    nc.vector.tensor_add(p_o[:, sl], xt_ps[NL:128, :],
                                         v_tiles[i][0:NL, sl])
                dsl = slice(i * LCW, (i + 1) * LCW)
                nc.sync.dma_start(wzn_d[:, dsl], wz_o[:])
                nc.sync.dma_start(pn_d[:, dsl], p_o[:])

    nc.compile()
    if not nc.is_finalized():
        nc.finalize()
    return nc


def _get_k1():
    global _K1
    if _K1 is None:
        _K1 = _build_k1()
    return _K1


def _get_k2():
    global _K2
    if _K2 is None:
        _K2 = _build_k2()
    return _K2


def _host_recurrence(G, Wx, sw, Gy, u, tss0, bz0):
    G = G.astype(np.float64)
    Wx = Wx.astype(np.float64)
    sw = sw.astype(np.float64)
    Gy = Gy.astype(np.float64)
    u = u.astype(np.float64)
    tss0 = tss0.astype(np.float64)
    bz0 = bz0.astype(np.float64)
    # basis order: index i < 64 is row i of P (resp. Cz), index 64 is xc0 (yc0)
    c = np.zeros(NV); c[NL] = 1.0
    d = np.zeros(NV); d[NL] = 1.0
    Cx = np.zeros((NL, NV)); Cy = np.zeros((NL, NV))
    u1v = np.zeros(NL); t2v = np.zeros(NL)
    u_new = np.zeros(NL); tss_new = np.zeros(NL); bz_new = np.zeros(NL)

    for i in range(NL):
        q = c @ (G @ c)
        a = c @ Wx[:, i]
        s = sw[i]
        qy = d @ (Gy @ d)
        ay = d @ Gy[:, i]
        sy = Gy[i, i]

        def one(uu):
            tz = (a + uu * q) / (np.sqrt(s + 2 * uu * a + uu * uu * q) + EPS)
            tssx = tss0[i] + tz * tz
            t = tz / np.sqrt(tssx)
            nrm = np.sqrt(sy + 2 * t * ay + t * t * qy)
            un = (ay + t * qy) / nrm
            return un, tz, t, tssx, nrm

        u1 = one(u[i])[0]
        u2, tz2, t2, tss2, nrm2 = one(u1)
        bzn = bz0[i] + u2 * tz2
        lam = (bzn / np.sqrt(tss2)) * t2 / nrm2

        Cx[i] = c; Cy[i] = d
        u1v[i] = u1; t2v[i] = t2
        u_new[i] = u2; tss_new[i] = tss2; bz_new[i] = bzn

        c = (1.0 - t2 * t2) * c; c[i] -= t2
        d = (1.0 - lam * t2) * d; d[i] -= lam

    return Cx, Cy, u1v, t2v, u_new, tss_new, bz_new


def kernel(x, y, mu_x, mu_y, u, Wz, Cz, t_sq_sum, bz, P, n):
    x = np.asarray(x, np.float32)
    y = np.asarray(y, np.float32)
    mu_x = np.asarray(mu_x, np.float32)
    mu_y = np.asarray(mu_y, np.float32)
    u = np.asarray(u, np.float32)
    Wz = np.asarray(Wz, np.float32)
    Cz = np.asarray(Cz, np.float32)
    t_sq_sum = np.asarray(t_sq_sum, np.float32)
    bz = np.asarray(bz, np.float32)
    P = np.asarray(P, np.float32)

    nf = np.float32(int(n))
    one = np.float32(1.0)
    mu_x_new = mu_x * (nf / (nf + one)) + x / (nf + one)
    mu_y_new = mu_y * (nf / (nf + one)) + y / (nf + one)
    xc0 = x - mu_x_new
    yc0 = y - mu_y_new

    # basis order everywhere: [P rows; xc0] / [Cz rows; yc0]
    Vy = np.ascontiguousarray(np.concatenate([Cz, yc0[None, :]], 0))
    core_ids = list(range(NCORES))
    v_shards = []
    wz_shards = []
    for k in core_ids:
        sl = slice(k * SH, (k + 1) * SH)
        v_shards.append(np.ascontiguousarray(
            np.concatenate([P[:, sl], xc0[None, sl]], 0)))
        wz_shards.append(np.ascontiguousarray(Wz[:, sl]))

    k1 = _get_k1()
    in1 = [{"v": v_shards[k], "wz": wz_shards[k], "vy": Vy} for k in core_ids]
    r1 = run_bass_kernel_spmd(k1, in1, core_ids)
    res1 = r1.results
    _PROFILE["k1_ns"] = r1.exec_time_ns

    gwx = np.sum([r["gwx"].astype(np.float64) for r in res1], axis=0)
    G = gwx[:, :NV]
    Wx = gwx[:, NV:]
    sw = np.sum([r["sw"].astype(np.float64)[:, 0] for r in res1], axis=0)
    Gy = res1[0]["gy"]

    Cx, Cy, u1v, t2v, u_new, tss_new, bz_new = _host_recurrence(
        G, Wx, sw, Gy, u, t_sq_sum, bz)

    A = np.concatenate([u1v[:, None] * Cx, t2v[:, None] * Cx], 0)  # [128, 65]
    aT = np.ascontiguousarray(A.T.astype(np.float32))              # [65, 128]
    ayT = np.ascontiguousarray((t2v[:, None] * Cy).T.astype(np.float32))

    k2 = _get_k2()
    in2 = [{"v": v_shards[k], "wz": wz_shards[k],
            "vy": np.ascontiguousarray(Vy[:, k * SHT:(k + 1) * SHT]),
            "at": aT, "ayt": ayT} for k in core_ids]
    r2 = run_bass_kernel_spmd(k2, in2, core_ids)
    res2 = r2.results
    _PROFILE["k2_ns"] = r2.exec_time_ns

    Wz_new = np.concatenate([r["wzn"] for r in res2], axis=1)
    P_new = np.concatenate([r["pn"] for r in res2], axis=1)
    Cz_new = np.concatenate([r["czn"] for r in res2], axis=1)

    return (mu_x_new, mu_y_new, u_new.astype(np.float32), Wz_new, Cz_new,
            tss_new.astype(np.float32), bz_new.astype(np.float32), P_new)


# revision 16
# speedup vs baseline: 1.0196x; 1.0196x over previous
"""IPLS.partial_fit Trainium2 kernel (8 NeuronCores).

Strategy: the 64-step latent recurrence only consumes a handful of scalars per
step (dots/norms of the evolving xc/yc against fixed rows of Wz/Cz/P).  Since
xc_i / yc_i stay inside span{xc0, P rows} / span{yc0, Cz rows}, every scalar is
a quadratic form of small Gram matrices:

  phase 1 (device, n_feat sharded 8-way): G=[xc0;P][xc0;P]^T, Wx=[xc0;P]Wz^T,
          sw=rownorm2(Wz) as per-core partials; Gy=[yc0;Cz][yc0;Cz]^T (full).
  phase 2 (host, microseconds): run the sequential recurrence on 65-vectors in
          fp64, producing coefficient matrices Cx, Cy and per-step scalars.
  phase 3 (device, sharded): rank-65 updates as matmuls:
          Wz_new = Wz + (diag(u1) Cx) V,  P_new = P + (diag(t2) Cx) V,
          Cz_new = Cz + (diag(t2) Cy) Vy.

No per-step collectives; both device kernels are memory-bound streams.
"""
import os
import sys

import numpy as np

for _p in ("/opt/trn_rl_repo", os.path.expanduser("~/.axon_site/_ro/trn_rl_repo")):
    if os.path.isdir(_p) and _p not in sys.path:
        sys.path.insert(0, _p)

import concourse.bass as bass  # noqa: E402
import concourse.bacc as bacc  # noqa: E402
import concourse.tile as tile  # noqa: E402
from concourse import mybir  # noqa: E402
from concourse.bass_utils import run_bass_kernel_spmd  # noqa: E402

NF = 131072
NT = 2048
NL = 64
NV = NL + 1
NCORES = 8
SH = NF // NCORES        # 16384 features per core
SHT = NT // NCORES       # 256 targets per core
NCHUNK = SH // 128       # 128 transpose chunks (phase 1)
NYCHUNK = NT // 128      # 16 y-side chunks
LOADCH = 16              # DMA load chunks for the big matrices
LCW = SH // LOADCH       # 1024 columns per load chunk
EPS = 1e-7
F32 = mybir.dt.float32

_K1 = None
_K2 = None
_PROFILE = {"k1_ns": None, "k2_ns": None}


def _new_bass():
    return bacc.Bacc("TRN2", target_bir_lowering=False, debug=False,
                     num_devices=NCORES)


def _build_k1():
    """Per core: V=[xc0_sh;P_sh] [65,SH], Wz_sh [64,SH], Vy=[yc0;Cz] [65,NT].
    Outputs: gwx [65,129] = [G | Wx] partials, sw [64,1] partial, gy [65,65]."""
    nc = _new_bass()
    v_d = nc.dram_tensor("v", [NV, SH], F32, kind="ExternalInput")
    wz_d = nc.dram_tensor("wz", [NL, SH], F32, kind="ExternalInput")
    vy_d = nc.dram_tensor("vy", [NV, NT], F32, kind="ExternalInput")
    gwx_d = nc.dram_tensor("gwx", [NV, NV + NL], F32, kind="ExternalOutput")
    gy_d = nc.dram_tensor("gy", [NV, NV], F32, kind="ExternalOutput")
    sw_d = nc.dram_tensor("sw", [NL, 1], F32, kind="ExternalOutput")
    ident_d = nc.inline_tensor(np.eye(128, dtype=np.float32), "ident")

    with tile.TileContext(nc) as tc:
        with tc.tile_pool(name="big", bufs=1) as big, \
             tc.tile_pool(name="work", bufs=4) as work, \
             tc.tile_pool(name="outp", bufs=1) as outp, \
             tc.tile_pool(name="psum", bufs=3, space="PSUM") as psum, \
             tc.tile_pool(name="acc", bufs=1, space="PSUM") as acc:

            id_sb = big.tile([128, 128], F32)
            nc.sync.dma_start(id_sb[:], ident_d[:])

            v_tiles = []
            wz_tiles = []
            for i in range(LOADCH):
                vt = big.tile([NV, LCW], F32, tag=f"v{i}")
                nc.sync.dma_start(vt[:], v_d[:, i * LCW:(i + 1) * LCW])
                v_tiles.append(vt)
                wt = big.tile([NL, LCW], F32, tag=f"w{i}")
                nc.sync.dma_start(wt[:], wz_d[:, i * LCW:(i + 1) * LCW])
                wz_tiles.append(wt)
            vy_sb = big.tile([NV, NT], F32)
            nc.sync.dma_start(vy_sb[:], vy_d[:])

            gwx_ps = acc.tile([NV, NV + NL], F32)
            gy_ps = acc.tile([NV, NV], F32)

            per_tile = LCW // 128  # 8 transpose chunks per load chunk
            for c in range(NCHUNK):
                vt_src = v_tiles[c // per_tile]
                wz_src = wz_tiles[c // per_tile]
                off = (c % per_tile) * 128
                sl = slice(off, off + 128)
                vt_ps = psum.tile([128, NV], F32, tag="vt")
                nc.tensor.transpose(vt_ps[:], vt_src[:, sl], id_sb[0:NV, 0:NV])
                wzt_ps = psum.tile([128, NL], F32, tag="wzt")
                nc.tensor.transpose(wzt_ps[:], wz_src[:, sl], id_sb[0:NL, 0:NL])
                vwt = work.tile([128, NV + NL], F32, tag="vwt")
                nc.vector.tensor_copy(vwt[:, 0:NV], vt_ps[:])
                nc.vector.tensor_copy(vwt[:, NV:NV + NL], wzt_ps[:])
                nc.tensor.matmul(gwx_ps[:], vwt[:, 0:NV], vwt[:],
                                 start=(c == 0), stop=(c == NCHUNK - 1))

            for c in range(NYCHUNK):
                sl = slice(c * 128, (c + 1) * 128)
                vyt_ps = psum.tile([128, NV], F32, tag="vt")
                nc.tensor.transpose(vyt_ps[:], vy_sb[:, sl], id_sb[0:NV, 0:NV])
                vyt = work.tile([128, NV], F32, tag="vyt")
                nc.vector.tensor_copy(vyt[:], vyt_ps[:])
                nc.tensor.matmul(gy_ps[:], vyt[:], vyt[:],
                                 start=(c == 0), stop=(c == NYCHUNK - 1))

            # sw = rowwise sum of Wz^2 (ACT engine, otherwise idle)
            sw_parts = outp.tile([NL, LOADCH], F32)
            for i in range(LOADCH):
                sq = work.tile([NL, LCW], F32, tag="sq")
                nc.scalar.activation(sq[:], wz_tiles[i][:],
                                     mybir.ActivationFunctionType.Square,
                                     accum_out=sw_parts[:, i:i + 1])
            sw_sb = outp.tile([NL, 1], F32)
            nc.vector.tensor_reduce(sw_sb[:], sw_parts[:],
                                    mybir.AxisListType.X, mybir.AluOpType.add)

            gwx_sb = outp.tile([NV, NV + NL], F32)
            nc.vector.tensor_copy(gwx_sb[:], gwx_ps[:])
            gy_sb = outp.tile([NV, NV], F32)
            nc.vector.tensor_copy(gy_sb[:], gy_ps[:])
            nc.sync.dma_start(gwx_d[:], gwx_sb[:])
            nc.sync.dma_start(gy_d[:], gy_sb[:])
            nc.sync.dma_start(sw_d[:], sw_sb[:])

    nc.compile()
    if not nc.is_finalized():
        nc.finalize()
    return nc


def _build_k2():
    """Per core: apply the rank-65 updates.
    out[0:64]  = Awz @ V -> Wz_new = Wz + .
    out[64:128]= Ap  @ V -> P_new  = P + .
    y-side: Cz_new = Cz + Acz @ Vy (target-sharded)."""
    nc = _new_bass()
    v_d = nc.dram_tensor("v", [NV, SH], F32, kind="ExternalInput")
    wz_d = nc.dram_tensor("wz", [NL, SH], F32, kind="ExternalInput")
    vy_d = nc.dram_tensor("vy", [NV, SHT], F32, kind="ExternalInput")
    at_d = nc.dram_tensor("at", [NV, 128], F32, kind="ExternalInput")
    ayt_d = nc.dram_tensor("ayt", [NV, NL], F32, kind="ExternalInput")
    wzn_d = nc.dram_tensor("wzn", [NL, SH], F32, kind="ExternalOutput")
    pn_d = nc.dram_tensor("pn", [NL, SH], F32, kind="ExternalOutput")
    czn_d = nc.dram_tensor("czn", [NL, SHT], F32, kind="ExternalOutput")

    with tile.TileContext(nc) as tc:
        with tc.tile_pool(name="big", bufs=1) as big, \
             tc.tile_pool(name="outs", bufs=4) as outs, \
             tc.tile_pool(name="psum", bufs=3, space="PSUM") as psum:

            at_sb = big.tile([NV, 128], F32)
            nc.sync.dma_start(at_sb[:], at_d[:])
            ayt_sb = big.tile([NV, NL], F32)
            nc.sync.dma_start(ayt_sb[:], ayt_d[:])
            vy_sb = big.tile([NV, SHT], F32)
            nc.sync.dma_start(vy_sb[:], vy_d[:])

            v_tiles = []
            wz_tiles = []
            for i in range(LOADCH):
                vt = big.tile([NV, LCW], F32, tag=f"v{i}")
                nc.sync.dma_start(vt[:], v_d[:, i * LCW:(i + 1) * LCW])
                v_tiles.append(vt)
                wt = big.tile([NL, LCW], F32, tag=f"w{i}")
                nc.sync.dma_start(wt[:], wz_d[:, i * LCW:(i + 1) * LCW])
                wz_tiles.append(wt)

            # y-side first (tiny); Vy rows are [Cz; yc0] so the Cz operand
            # of the add starts at partition 0
            yt_ps = psum.tile([NL, SHT], F32, tag="ypsum")
            nc.tensor.matmul(yt_ps[:], ayt_sb[:], vy_sb[:], start=True, stop=True)
            cz_o = outs.tile([NL, SHT], F32, tag="czo")
            nc.vector.tensor_add(cz_o[:], yt_ps[:], vy_sb[0:NL, :])
            nc.sync.dma_start(czn_d[:], cz_o[:])

            per_tile = LCW // 512  # 2 matmul slices per load chunk
            for i in range(LOADCH):
                wz_o = outs.tile([NL, LCW], F32, tag="wzo")
                p_o = outs.tile([NL, LCW], F32, tag="po")
                for j in range(per_tile):
                    sl = slice(j * 512, (j + 1) * 512)
                    xt_ps = psum.tile([128, 512], F32, tag="xpsum")
                    nc.tensor.matmul(xt_ps[:], at_sb[:], v_tiles[i][:, sl],
                                     start=True, stop=True)
                    nc.vector.tensor_add(wz_o[:, sl], xt_ps[0:NL, :],
                                         wz_tiles[i][:, sl])
                    nc.vector.tensor_add(p_o[:, sl], xt_ps[NL:128, :],
                                         v_tiles[i][0:NL, sl])
                dsl = slice(i * LCW, (i + 1) * LCW)
                nc.sync.dma_start(wzn_d[:, dsl], wz_o[:])
                nc.sync.dma_start(pn_d[:, dsl], p_o[:])

    nc.compile()
    if not nc.is_finalized():
        nc.finalize()
    return nc


def _get_k1():
    global _K1
    if _K1 is None:
        _K1 = _build_k1()
    return _K1


def _get_k2():
    global _K2
    if _K2 is None:
        _K2 = _build_k2()
    return _K2


def _host_recurrence(G, Wx, sw, Gy, u, tss0, bz0):
    G = G.astype(np.float64)
    Wx = Wx.astype(np.float64)
    sw = sw.astype(np.float64)
    Gy = Gy.astype(np.float64)
    u = u.astype(np.float64)
    tss0 = tss0.astype(np.float64)
    bz0 = bz0.astype(np.float64)
    # basis order: index i < 64 is row i of P (resp. Cz), index 64 is xc0 (yc0)
    c = np.zeros(NV); c[NL] = 1.0
    d = np.zeros(NV); d[NL] = 1.0
    Cx = np.zeros((NL, NV)); Cy = np.zeros((NL, NV))
    u1v = np.zeros(NL); t2v = np.zeros(NL)
    u_new = np.zeros(NL); tss_new = np.zeros(NL); bz_new = np.zeros(NL)

    for i in range(NL):
        q = c @ (G @ c)
        a = c @ Wx[:, i]
        s = sw[i]
        qy = d @ (Gy @ d)
        ay = d @ Gy[:, i]
        sy = Gy[i, i]

        def one(uu):
            tz = (a + uu * q) / (np.sqrt(s + 2 * uu * a + uu * uu * q) + EPS)
            tssx = tss0[i] + tz * tz
            t = tz / np.sqrt(tssx)
            nrm = np.sqrt(sy + 2 * t * ay + t * t * qy)
            un = (ay + t * qy) / nrm
            return un, tz, t, tssx, nrm

        u1 = one(u[i])[0]
        u2, tz2, t2, tss2, nrm2 = one(u1)
        bzn = bz0[i] + u2 * tz2
        lam = (bzn / np.sqrt(tss2)) * t2 / nrm2

        Cx[i] = c; Cy[i] = d
        u1v[i] = u1; t2v[i] = t2
        u_new[i] = u2; tss_new[i] = tss2; bz_new[i] = bzn

        c = (1.0 - t2 * t2) * c; c[i] -= t2
        d = (1.0 - lam * t2) * d; d[i] -= lam

    return Cx, Cy, u1v, t2v, u_new, tss_new, bz_new


def kernel(x, y, mu_x, mu_y, u, Wz, Cz, t_sq_sum, bz, P, n):
    x = np.asarray(x, np.float32)
    y = np.asarray(y, np.float32)
    mu_x = np.asarray(mu_x, np.float32)
    mu_y = np.asarray(mu_y, np.float32)
    u = np.asarray(u, np.float32)
    Wz = np.asarray(Wz, np.float32)
    Cz = np.asarray(Cz, np.float32)
    t_sq_sum = np.asarray(t_sq_sum, np.float32)
    bz = np.asarray(bz, np.float32)
    P = np.asarray(P, np.float32)

    nf = np.float32(int(n))
    one = np.float32(1.0)
    mu_x_new = mu_x * (nf / (nf + one)) + x / (nf + one)
    mu_y_new = mu_y * (nf / (nf + one)) + y / (nf + one)
    xc0 = x - mu_x_new
    yc0 = y - mu_y_new

    # basis order everywhere: [P rows; xc0] / [Cz rows; yc0]
    Vy = np.ascontiguousarray(np.concatenate([Cz, yc0[None, :]], 0))
    core_ids = list(range(NCORES))
    v_shards = []
    wz_shards = []
    for k in core_ids:
        sl = slice(k * SH, (k + 1) * SH)
        v_shards.append(np.ascontiguousarray(
            np.concatenate([P[:, sl], xc0[None, sl]], 0)))
        wz_shards.append(np.ascontiguousarray(Wz[:, sl]))

    k1 = _get_k1()
    in1 = [{"v": v_shards[k], "wz": wz_shards[k], "vy": Vy} for k in core_ids]
    r1 = run_bass_kernel_spmd(k1, in1, core_ids)
    res1 = r1.results
    _PROFILE["k1_ns"] = r1.exec_time_ns

    gwx = np.sum([r["gwx"].astype(np.float64) for r in res1], axis=0)
    G = gwx[:, :NV]
    Wx = gwx[:, NV:]
    sw = np.sum([r["sw"].astype(np.float64)[:, 0] for r in res1], axis=0)
    Gy = res1[0]["gy"]

    Cx, Cy, u1v, t2v, u_new, tss_new, bz_new = _host_recurrence(
        G, Wx, sw, Gy, u, t_sq_sum, bz)

    A = np.concatenate([u1v[:, None] * Cx, t2v[:, None] * Cx], 0)  # [128, 65]
    aT = np.ascontiguousarray(A.T.astype(np.float32))              # [65, 128]
    ayT = np.ascontiguousarray((t2v[:, None] * Cy).T.astype(np.float32))

    k2 = _get_k2()
    in2 = [{"v": v_shards[k], "wz": wz_shards[k],
            "vy": np.ascontiguousarray(Vy[:, k * SHT:(k + 1) * SHT]),
            "at": aT, "ayt": ayT} for k in core_ids]
    r2 = run_bass_kernel_spmd(k2, in2, core_ids)
    res2 = r2.results
    _PROFILE["k2_ns"] = r2.exec_time_ns

    Wz_new = np.concatenate([r["wzn"] for r in res2], axis=1)
    P_new = np.concatenate([r["pn"] for r in res2], axis=1)
    Cz_new = np.concatenate([r["czn"] for r in res2], axis=1)

    return (mu_x_new, mu_y_new, u_new.astype(np.float32), Wz_new, Cz_new,
            tss_new.astype(np.float32), bz_new.astype(np.float32), P_new)


# revision 19
# speedup vs baseline: 1.0328x; 1.0129x over previous
"""IPLS.partial_fit Trainium2 kernel (8 NeuronCores).

Strategy: the 64-step latent recurrence only consumes a handful of scalars per
step (dots/norms of the evolving xc/yc against fixed rows of Wz/Cz/P).  Since
xc_i / yc_i stay inside span{xc0, P rows} / span{yc0, Cz rows}, every scalar is
a quadratic form of small Gram matrices:

  phase 1 (device, n_feat sharded 8-way): G=[xc0;P][xc0;P]^T, Wx=[xc0;P]Wz^T,
          sw=rownorm2(Wz) as per-core partials; Gy=[yc0;Cz][yc0;Cz]^T (full).
  phase 2 (host, microseconds): run the sequential recurrence on 65-vectors in
          fp64, producing coefficient matrices Cx, Cy and per-step scalars.
  phase 3 (device, sharded): rank-65 updates as matmuls:
          Wz_new = Wz + (diag(u1) Cx) V,  P_new = P + (diag(t2) Cx) V,
          Cz_new = Cz + (diag(t2) Cy) Vy.

No per-step collectives; both device kernels are memory-bound streams.
"""
import os
import sys

import numpy as np

for _p in ("/opt/trn_rl_repo", os.path.expanduser("~/.axon_site/_ro/trn_rl_repo")):
    if os.path.isdir(_p) and _p not in sys.path:
        sys.path.insert(0, _p)

import concourse.bass as bass  # noqa: E402
import concourse.bacc as bacc  # noqa: E402
import concourse.tile as tile  # noqa: E402
from concourse import mybir  # noqa: E402
from concourse.bass_utils import run_bass_kernel_spmd  # noqa: E402

NF = 131072
NT = 2048
NL = 64
NV = NL + 1
NCORES = 8
SH = NF // NCORES        # 16384 features per core
SHT = NT // NCORES       # 256 targets per core
NCHUNK = SH // 128       # 128 transpose chunks (phase 1)
NYCHUNK = NT // 128      # 16 y-side chunks
LOADCH = 16              # DMA load chunks for the big matrices
LCW = SH // LOADCH       # 1024 columns per load chunk
EPS = 1e-7
F32 = mybir.dt.float32

_K1 = None
_K2 = None
_PROFILE = {"k1_ns": None, "k2_ns": None}


def _new_bass():
    return bacc.Bacc("TRN2", target_bir_lowering=False, debug=False,
                     num_devices=NCORES)


def _build_k1():
    """Per core: V=[xc0_sh;P_sh] [65,SH], Wz_sh [64,SH], Vy=[yc0;Cz] [65,NT].
    Outputs: gwx [65,129] = [G | Wx] partials, sw [64,1] partial, gy [65,65]."""
    nc = _new_bass()
    v_d = nc.dram_tensor("v", [NV, SH], F32, kind="ExternalInput")
    wz_d = nc.dram_tensor("wz", [NL, SH], F32, kind="ExternalInput")
    vy_d = nc.dram_tensor("vy", [NV, NT], F32, kind="ExternalInput")
    gwx_d = nc.dram_tensor("gwx", [NV, NV + NL], F32, kind="ExternalOutput")
    gy_d = nc.dram_tensor("gy", [NV, NV], F32, kind="ExternalOutput")
    sw_d = nc.dram_tensor("sw", [NL, 1], F32, kind="ExternalOutput")
    ident_d = nc.inline_tensor(np.eye(128, dtype=np.float32), "ident")

    with tile.TileContext(nc) as tc:
        with tc.tile_pool(name="big", bufs=1) as big, \
             tc.tile_pool(name="work", bufs=4) as work, \
             tc.tile_pool(name="outp", bufs=1) as outp, \
             tc.tile_pool(name="psum", bufs=3, space="PSUM") as psum, \
             tc.tile_pool(name="acc", bufs=1, space="PSUM") as acc:

            id_sb = big.tile([128, 128], F32)
            nc.sync.dma_start(id_sb[:], ident_d[:])

            v_tiles = []
            wz_tiles = []
            for i in range(LOADCH):
                vt = big.tile([NV, LCW], F32, tag=f"v{i}")
                nc.sync.dma_start(vt[:], v_d[:, i * LCW:(i + 1) * LCW])
                v_tiles.append(vt)
                wt = big.tile([NL, LCW], F32, tag=f"w{i}")
                nc.sync.dma_start(wt[:], wz_d[:, i * LCW:(i + 1) * LCW])
                wz_tiles.append(wt)
            vy_sb = big.tile([NV, NT], F32)
            nc.sync.dma_start(vy_sb[:], vy_d[:])

            gwx_ps = acc.tile([NV, NV + NL], F32)
            gy_ps = acc.tile([NV, NV], F32)

            per_tile = LCW // 128  # 8 transpose chunks per load chunk
            for c in range(NCHUNK):
                vt_src = v_tiles[c // per_tile]
                wz_src = wz_tiles[c // per_tile]
                off = (c % per_tile) * 128
                sl = slice(off, off + 128)
                vt_ps = psum.tile([128, NV], F32, tag="vt")
                nc.tensor.transpose(vt_ps[:], vt_src[:, sl], id_sb[0:NV, 0:NV])
                wzt_ps = psum.tile([128, NL], F32, tag="wzt")
                nc.tensor.transpose(wzt_ps[:], wz_src[:, sl], id_sb[0:NL, 0:NL])
                vwt = work.tile([128, NV + NL], F32, tag="vwt")
                nc.vector.tensor_copy(vwt[:, 0:NV], vt_ps[:])
                nc.vector.tensor_copy(vwt[:, NV:NV + NL], wzt_ps[:])
                nc.tensor.matmul(gwx_ps[:], vwt[:, 0:NV], vwt[:],
                                 start=(c == 0), stop=(c == NCHUNK - 1))

            for c in range(NYCHUNK):
                sl = slice(c * 128, (c + 1) * 128)
                vyt_ps = psum.tile([128, NV], F32, tag="vt")
                nc.tensor.transpose(vyt_ps[:], vy_sb[:, sl], id_sb[0:NV, 0:NV])
                vyt = work.tile([128, NV], F32, tag="vyt")
                nc.vector.tensor_copy(vyt[:], vyt_ps[:])
                nc.tensor.matmul(gy_ps[:], vyt[:], vyt[:],
                                 start=(c == 0), stop=(c == NYCHUNK - 1))

            # sw = rowwise sum of Wz^2 (ACT engine, otherwise idle)
            sw_parts = outp.tile([NL, LOADCH], F32)
            for i in range(LOADCH):
                sq = work.tile([NL, LCW], F32, tag="sq")
                nc.scalar.activation(sq[:], wz_tiles[i][:],
                                     mybir.ActivationFunctionType.Square,
                                     accum_out=sw_parts[:, i:i + 1])
            sw_sb = outp.tile([NL, 1], F32)
            nc.vector.tensor_reduce(sw_sb[:], sw_parts[:],
                                    mybir.AxisListType.X, mybir.AluOpType.add)

            gwx_sb = outp.tile([NV, NV + NL], F32)
            nc.vector.tensor_copy(gwx_sb[:], gwx_ps[:])
            gy_sb = outp.tile([NV, NV], F32)
            nc.vector.tensor_copy(gy_sb[:], gy_ps[:])
            nc.sync.dma_start(gwx_d[:], gwx_sb[:])
            nc.sync.dma_start(gy_d[:], gy_sb[:])
            nc.sync.dma_start(sw_d[:], sw_sb[:])

    nc.compile()
    if not nc.is_finalized():
        nc.finalize()
    return nc


def _build_k2():
    """Per core: apply the rank-65 updates.
    out[0:64]  = Awz @ V -> Wz_new = Wz + .
    out[64:128]= Ap  @ V -> P_new  = P + .
    y-side: Cz_new = Cz + Acz @ Vy (target-sharded)."""
    nc = _new_bass()
    v_d = nc.dram_tensor("v", [NV, SH], F32, kind="ExternalInput")
    wz_d = nc.dram_tensor("wz", [NL, SH], F32, kind="ExternalInput")
    vy_d = nc.dram_tensor("vy", [NV, SHT], F32, kind="ExternalInput")
    at_d = nc.dram_tensor("at", [NV, 128], F32, kind="ExternalInput")
    ayt_d = nc.dram_tensor("ayt", [NV, NL], F32, kind="ExternalInput")
    wzn_d = nc.dram_tensor("wzn", [NL, SH], F32, kind="ExternalOutput")
    pn_d = nc.dram_tensor("pn", [NL, SH], F32, kind="ExternalOutput")
    czn_d = nc.dram_tensor("czn", [NL, SHT], F32, kind="ExternalOutput")

    with tile.TileContext(nc) as tc:
        with tc.tile_pool(name="big", bufs=1) as big, \
             tc.tile_pool(name="outs", bufs=4) as outs, \
             tc.tile_pool(name="psum", bufs=4, space="PSUM") as psum:

            at_sb = big.tile([NV, 128], F32)
            nc.sync.dma_start(at_sb[:], at_d[:])
            ayt_sb = big.tile([NV, NL], F32)
            nc.sync.dma_start(ayt_sb[:], ayt_d[:])
            vy_sb = big.tile([NV, SHT], F32)
            nc.sync.dma_start(vy_sb[:], vy_d[:])

            v_tiles = []
            wz_tiles = []
            for i in range(LOADCH):
                vt = big.tile([NV, LCW], F32, tag=f"v{i}")
                nc.sync.dma_start(vt[:], v_d[:, i * LCW:(i + 1) * LCW])
                v_tiles.append(vt)
                wt = big.tile([NL, LCW], F32, tag=f"w{i}")
                nc.sync.dma_start(wt[:], wz_d[:, i * LCW:(i + 1) * LCW])
                wz_tiles.append(wt)

            # y-side first (tiny); Vy rows are [Cz; yc0] so the Cz operand
            # of the add starts at partition 0
            yt_ps = psum.tile([NL, SHT], F32, tag="ypsum")
            nc.tensor.matmul(yt_ps[:], ayt_sb[:], vy_sb[:], start=True, stop=True)
            cz_o = outs.tile([NL, SHT], F32, tag="czo")
            nc.vector.tensor_add(cz_o[:], yt_ps[:], vy_sb[0:NL, :])
            nc.sync.dma_start(czn_d[:], cz_o[:])

            per_tile = LCW // 512  # 2 matmul slices per load chunk
            for i in range(LOADCH):
                wz_o = outs.tile([NL, LCW], F32, tag="wzo")
                p_o = outs.tile([NL, LCW], F32, tag="po")
                for j in range(per_tile):
                    sl = slice(j * 512, (j + 1) * 512)
                    xt_ps = psum.tile([128, 512], F32, tag="xpsum")
                    nc.tensor.matmul(xt_ps[:], at_sb[:], v_tiles[i][:, sl],
                                     start=True, stop=True)
                    nc.vector.tensor_add(wz_o[:, sl], xt_ps[0:NL, :],
                                         wz_tiles[i][:, sl])
                    nc.vector.tensor_add(p_o[:, sl], xt_ps[NL:128, :],
                                         v_tiles[i][0:NL, sl])
                dsl = slice(i * LCW, (i + 1) * LCW)
                nc.sync.dma_start(wzn_d[:, dsl], wz_o[:])
                nc.sync.dma_start(pn_d[:, dsl], p_o[:])

    nc.compile()
    if not nc.is_finalized():
        nc.finalize()
    return nc


def _get_k1():
    global _K1
    if _K1 is None:
        _K1 = _build_k1()
    return _K1


def _get_k2():
    global _K2
    if _K2 is None:
        _K2 = _build_k2()
    return _K2


def _host_recurrence(G, Wx, sw, Gy, u, tss0, bz0):
    G = G.astype(np.float64)
    Wx = Wx.astype(np.float64)
    sw = sw.astype(np.float64)
    Gy = Gy.astype(np.float64)
    u = u.astype(np.float64)
    tss0 = tss0.astype(np.float64)
    bz0 = bz0.astype(np.float64)
    # basis order: index i < 64 is row i of P (resp. Cz), index 64 is xc0 (yc0)
    c = np.zeros(NV); c[NL] = 1.0
    d = np.zeros(NV); d[NL] = 1.0
    Cx = np.zeros((NL, NV)); Cy = np.zeros((NL, NV))
    u1v = np.zeros(NL); t2v = np.zeros(NL)
    u_new = np.zeros(NL); tss_new = np.zeros(NL); bz_new = np.zeros(NL)

    for i in range(NL):
        q = c @ (G @ c)
        a = c @ Wx[:, i]
        s = sw[i]
        qy = d @ (Gy @ d)
        ay = d @ Gy[:, i]
        sy = Gy[i, i]

        def one(uu):
            tz = (a + uu * q) / (np.sqrt(s + 2 * uu * a + uu * uu * q) + EPS)
            tssx = tss0[i] + tz * tz
            t = tz / np.sqrt(tssx)
            nrm = np.sqrt(sy + 2 * t * ay + t * t * qy)
            un = (ay + t * qy) / nrm
            return un, tz, t, tssx, nrm

        u1 = one(u[i])[0]
        u2, tz2, t2, tss2, nrm2 = one(u1)
        bzn = bz0[i] + u2 * tz2
        lam = (bzn / np.sqrt(tss2)) * t2 / nrm2

        Cx[i] = c; Cy[i] = d
        u1v[i] = u1; t2v[i] = t2
        u_new[i] = u2; tss_new[i] = tss2; bz_new[i] = bzn

        c = (1.0 - t2 * t2) * c; c[i] -= t2
        d = (1.0 - lam * t2) * d; d[i] -= lam

    return Cx, Cy, u1v, t2v, u_new, tss_new, bz_new


def kernel(x, y, mu_x, mu_y, u, Wz, Cz, t_sq_sum, bz, P, n):
    x = np.asarray(x, np.float32)
    y = np.asarray(y, np.float32)
    mu_x = np.asarray(mu_x, np.float32)
    mu_y = np.asarray(mu_y, np.float32)
    u = np.asarray(u, np.float32)
    Wz = np.asarray(Wz, np.float32)
    Cz = np.asarray(Cz, np.float32)
    t_sq_sum = np.asarray(t_sq_sum, np.float32)
    bz = np.asarray(bz, np.float32)
    P = np.asarray(P, np.float32)

    nf = np.float32(int(n))
    one = np.float32(1.0)
    mu_x_new = mu_x * (nf / (nf + one)) + x / (nf + one)
    mu_y_new = mu_y * (nf / (nf + one)) + y / (nf + one)
    xc0 = x - mu_x_new
    yc0 = y - mu_y_new

    # basis order everywhere: [P rows; xc0] / [Cz rows; yc0]
    Vy = np.ascontiguousarray(np.concatenate([Cz, yc0[None, :]], 0))
    core_ids = list(range(NCORES))
    v_shards = []
    wz_shards = []
    for k in core_ids:
        sl = slice(k * SH, (k + 1) * SH)
        v_shards.append(np.ascontiguousarray(
            np.concatenate([P[:, sl], xc0[None, sl]], 0)))
        wz_shards.append(np.ascontiguousarray(Wz[:, sl]))

    k1 = _get_k1()
    in1 = [{"v": v_shards[k], "wz": wz_shards[k], "vy": Vy} for k in core_ids]
    r1 = run_bass_kernel_spmd(k1, in1, core_ids)
    res1 = r1.results
    _PROFILE["k1_ns"] = r1.exec_time_ns

    gwx = np.sum([r["gwx"].astype(np.float64) for r in res1], axis=0)
    G = gwx[:, :NV]
    Wx = gwx[:, NV:]
    sw = np.sum([r["sw"].astype(np.float64)[:, 0] for r in res1], axis=0)
    Gy = res1[0]["gy"]

    Cx, Cy, u1v, t2v, u_new, tss_new, bz_new = _host_recurrence(
        G, Wx, sw, Gy, u, t_sq_sum, bz)

    A = np.concatenate([u1v[:, None] * Cx, t2v[:, None] * Cx], 0)  # [128, 65]
    aT = np.ascontiguousarray(A.T.astype(np.float32))              # [65, 128]
    ayT = np.ascontiguousarray((t2v[:, None] * Cy).T.astype(np.float32))

    k2 = _get_k2()
    in2 = [{"v": v_shards[k], "wz": wz_shards[k],
            "vy": np.ascontiguousarray(Vy[:, k * SHT:(k + 1) * SHT]),
            "at": aT, "ayt": ayT} for k in core_ids]
    r2 = run_bass_kernel_spmd(k2, in2, core_ids)
    res2 = r2.results
    _PROFILE["k2_ns"] = r2.exec_time_ns

    Wz_new = np.concatenate([r["wzn"] for r in res2], axis=1)
    P_new = np.concatenate([r["pn"] for r in res2], axis=1)
    Cz_new = np.concatenate([r["czn"] for r in res2], axis=1)

    return (mu_x_new, mu_y_new, u_new.astype(np.float32), Wz_new, Cz_new,
            tss_new.astype(np.float32), bz_new.astype(np.float32), P_new)
